# revision 21
# baseline (speedup 1.0000x reference)
"""Bahdanau attention kernel for 8 Trainium2 NeuronCores.

Problem shapes (hardcoded): hidden [2, 32, 1024], encoder_outputs [32, 2048, 1024],
Wq/Wk [1024, 1024], bq/bk/wv [1024], bv scalar. Output [32, 1, 1024].

Sharding: data-parallel over batch B=32 -> 4 batches per core, weights replicated.
bv is dropped entirely (softmax is invariant to constant shifts).

Key structure (v3):
- The K-projection (enc @ Wk.T, the dominant 137 GFLOP) runs in fp8e4 with
  MatmulPerfMode.DoubleRow (2 fp8 MACs per cell per cycle): 4 accumulating MMs
  of contraction 256 per (o-tile, s-chunk). Wk is pre-scaled by 64 on the host
  so its values sit in fp8's normal range; the inverse scale folds into the
  tanh activation's free scale multiplier.
- All layout/dtype prep happens host-side in make_in_maps (sharding code):
  enc ships twice (pre-transposed fp8 [h, s] tiles for the projection, natural
  bf16 rows for the einsum); Wk^T/Wq^T/hid^T/biases ship pre-transposed. No
  on-device casts or transposes; staging is chunk-granular DMA in deep rings.
- The q+bq+bk bias folds into the tanh as a per-partition bias while the
  activation reads the matmul PSUM directly.
- The wv contraction over h runs on the vector engine (broadcast multiply +
  half-tree sum over the 8 o-tiles), leaving the PE a single ones-weight
  matmul per chunk for the cross-partition sum: the PE's M=1 matmul count
  drops from 256 to 144 per core.
- scores never materialize: exp() is applied per chunk straight from the
  scores PSUM (no max-shift needed: |scores| <= sum|wv| <= 16), and the
  attn @ enc einsum accumulates per chunk with unnormalized weights; only the
  final [1, H] row is scaled by 1/sum.
"""

from contextlib import ExitStack

import numpy as np

import concourse.bacc as bacc
import concourse.bass as bass
import concourse.mybir as mybir
import concourse.tile as tile
from concourse.bass_utils import run_bass_kernel_spmd

B, S, H = 32, 2048, 1024
NCORES = 8
BPC = B // NCORES  # 4 batches per core
F32 = mybir.dt.float32
BF16 = mybir.dt.bfloat16
FP8 = mybir.dt.float8e4
HT = H // 128  # 8 chunks of 128 along h or o
ST = S // 128  # 16 s-tiles of 128
SC = S // 512  # 4 s-chunks of 512
KT = 4  # fp8 DoubleRow: 4 contraction steps of 256
WK_SCALE = 64.0
Tanh = mybir.ActivationFunctionType.Tanh
Exp = mybir.ActivationFunctionType.Exp
X = mybir.AxisListType.X
DR = mybir.MatmulPerfMode.DoubleRow
Mult = mybir.AluOpType.mult
Add = mybir.AluOpType.add

ts = bass.ts


def build_program():
    nc = bacc.Bacc("TRN2", target_bir_lowering=False, debug=False)

    # enc^T fp8 tiles: encT8[b, j, p, c, s] = fp8(enc[b, 512j+s, 128c+p])
    encT8_d = nc.dram_tensor("encT8", [BPC, SC, 128, HT, 512], FP8, kind="ExternalInput")
    # enc^T bf16 tiles (einsum operand): encT16[b, j, p, c, s]
    encT16_d = nc.dram_tensor("encT16", [BPC, SC, 128, HT, 512], BF16, kind="ExternalInput")
    # Wk^T fp8 (x64): wkT8[p, i, c, m] = fp8(64 * Wk[128i+m, 128c+p])
    wkT8_d = nc.dram_tensor("wkT8", [128, HT, HT, 128], FP8, kind="ExternalInput")
    # Wq^T bf16: wqT16[p, t, c, n] = bf16(Wq[128t+n, 128c+p])
    wqT16_d = nc.dram_tensor("wqT16", [128, HT, HT, 128], BF16, kind="ExternalInput")
    # hid^T bf16: hidT[p, c, b] = bf16(hidden[-1][4c0+b? no: hid[b, 128c+p]])
    hidT_d = nc.dram_tensor("hidT", [128, HT, BPC], BF16, kind="ExternalInput")
    bqkT_d = nc.dram_tensor("bqkT", [128, HT], F32, kind="ExternalInput")  # (bq+bk)^T
    wvT_d = nc.dram_tensor("wvT", [128, HT], F32, kind="ExternalInput")  # wv^T
    outT_d = nc.dram_tensor("outT", [BPC, 128, HT], F32, kind="ExternalOutput")

    with tile.TileContext(nc) as tc, ExitStack() as ctx:
        consts = ctx.enter_context(tc.tile_pool(name="consts", bufs=1))
        tp = ctx.enter_context(tc.tile_pool(name="tp", bufs=2, space="PSUM"))
        kp = ctx.enter_context(tc.tile_pool(name="kp", bufs=4, space="PSUM"))
        bcp = ctx.enter_context(tc.tile_pool(name="bcp", bufs=2, space="PSUM"))
        # chunk-granular staging rings
        encT_p = ctx.enter_context(tc.tile_pool(name="encT", bufs=6))  # 512KB/slot
        encbf = ctx.enter_context(tc.tile_pool(name="encbf", bufs=7))  # 1MB/slot
        eT_p = ctx.enter_context(tc.tile_pool(name="eT", bufs=2))
        batch = ctx.enter_context(tc.tile_pool(name="batch", bufs=1))

        # ---- staging helpers (chunk granular, plain HWDGE DMAs) ----
        def load_enc_chunk(b, j):
            eT8 = encT_p.tile([128, HT, 512], FP8, tag="encT8")
            nc.sync.dma_start(eT8[:], encT8_d[b, j])
            eT16 = encbf.tile([128, HT, 512], BF16, tag="encbf")
            nc.sync.dma_start(eT16[:], encT16_d[b, j])
            return eT8, eT16

        # ---- weights + small consts (pre-transposed on host) ----
        wkT8 = consts.tile([128, HT, HT, 128], FP8, tag="wkT8")
        nc.sync.dma_start(wkT8[:], wkT8_d[:])

        bqkT = consts.tile([128, HT], F32, tag="bqkT")
        nc.scalar.dma_start(bqkT[:], bqkT_d[:])
        wvT = consts.tile([128, HT], F32, tag="wvT")
        nc.scalar.dma_start(wvT[:], wvT_d[:])
        hidT = consts.tile([128, HT, BPC], BF16, tag="hidT")
        nc.scalar.dma_start(hidT[:], hidT_d[:])
        ones_bf = consts.tile([1, 128], BF16, tag="ones")
        nc.vector.memset(ones_bf[:], 1.0)
        ones_col = consts.tile([128, 1], BF16, tag="onescol")
        nc.vector.memset(ones_col[:], 1.0)
        ones_f32 = consts.tile([1, 128], F32, tag="onesf32")
        nc.vector.memset(ones_f32[:], 1.0)
        wvT16 = consts.tile([128, HT], BF16, tag="wvT16")
        nc.vector.tensor_copy(wvT16[:], wvT[:])
        wqT16 = consts.tile([128, HT, HT, 128], BF16, tag="wqT16")
        nc.scalar.dma_start(wqT16[:], wqT16_d[:])

        staged = {}
        for j in range(SC):
            staged[(0, j)] = load_enc_chunk(0, j)

        def kproj_mm_chain(i, eT8):
            pk = kp.tile([128, 512], F32, tag="kp", name="pk")
            for kt in range(KT):
                nc.tensor.matmul(
                    pk[:],
                    wkT8[:, i, ts(kt, 2), :],
                    eT8[:, ts(kt, 2), :],
                    start=(kt == 0),
                    stop=(kt == KT - 1),
                    perf_mode=DR,
                )
            return pk

        # ---- chunk (0,0): emit the first 4 kproj MM chains ahead of the
        # q-projection, so the PE starts as soon as wkT8 + the first enc tile
        # land (the q-projection waits on the larger wqT16 transfer).
        eT8_00, eT16_00 = staged.pop((0, 0))
        pks00 = [kproj_mm_chain(i, eT8_00) for i in range(4)]

        # ---- q^T + bq + bk: qkb[o(part), o-chunk t, b] (bf16 MMs, fp32 out;
        # pq lives in the tp pool so the kp ring stays free for pks00) ----
        qkb = consts.tile([128, HT, BPC], F32, tag="qkb")
        for t in range(HT):
            pq = tp.tile([128, BPC], F32, tag="tp", name="pq")
            for c in range(HT):
                nc.tensor.matmul(
                    pq[:],
                    wqT16[:, t, c, :],
                    hidT[:, c, :],
                    start=(c == 0),
                    stop=(c == HT - 1),
                )
            nc.vector.tensor_scalar_add(qkb[:, t, :], pq[:], bqkT[:, t : t + 1])

        # ---- per-batch pieces ----
        # The chunk tail is software-pipelined one chunk behind the
        # K-projection: the scores cross-partition MM + exp for chunk j are
        # emitted mid-way through chunk j+1's kproj (so the exp latency hides
        # under the remaining MM chains), and the attn transposes + einsum
        # follow the full head.
        def kproj_head(b, j, eT8, pks_pre=(), mid_cb=None, skip_reduce=False):
            # K^T tiles (fp8 DoubleRow) + fused bias/tanh (undoes the x64
            # weight prescale) -> eT_j[o(part), o-chunk i, s(512)]
            eT_j = eT_p.tile([128, HT, 512], BF16, tag="eTj")
            for i in range(HT):
                pk = pks_pre[i] if i < len(pks_pre) else kproj_mm_chain(i, eT8)
                nc.scalar.activation(
                    eT_j[:, i, :],
                    pk[:],
                    Tanh,
                    bias=qkb[:, i, b : b + 1],
                    scale=1.0 / WK_SCALE,
                )
                if i == 3 and mid_cb is not None:
                    mid_cb()
            if skip_reduce:
                return eT_j, None

            # scores chunk j = wv . eT_j: per-o-tile multiply by wv^T and
            # half-tree sum over the 8 o-tiles, all on DVE.
            etw = batch.tile([128, HT, 512], BF16, tag="etw", bufs=2)
            for i in range(HT):
                nc.vector.tensor_scalar_mul(
                    etw[:, i, :], eT_j[:, i, :], wvT[:, i : i + 1]
                )
            th1 = batch.tile([128, 4, 512], BF16, tag="th1", bufs=2)
            nc.vector.tensor_add(th1[:], etw[:, 0:4, :], etw[:, 4:8, :])
            th2 = batch.tile([128, 2, 512], BF16, tag="th2", bufs=2)
            nc.vector.tensor_add(th2[:], th1[:, 0:2, :], th1[:, 2:4, :])
            th3 = batch.tile([128, 512], BF16, tag="th3", bufs=2)
            nc.vector.tensor_add(th3[:], th2[:, 0, :], th2[:, 1, :])
            return eT_j, th3

        def tail_scores(j, th3, ssum4):
            # cross-partition sum of the wv-weighted tanh -> scores PSUM row
            ps = kp.tile([1, 512], F32, tag="kp", name="ps")
            nc.tensor.matmul(ps[:], ones_col[:], th3[:], start=True, stop=True)
            expj = batch.tile([1, 512], BF16, tag="expj", bufs=2)
            nc.scalar.activation(
                expj[:], ps[:], Exp, accum_out=ssum4[0:1, j : j + 1]
            )
            return expj

        def tail_scores_direct(j, eT_j, ssum4):
            # final-chunk fast path: contract wv on the PE (no DVE wait)
            ps = kp.tile([1, 512], F32, tag="kp", name="psd")
            for i in range(HT):
                nc.tensor.matmul(
                    ps[:],
                    wvT16[:, i : i + 1],
                    eT_j[:, i, :],
                    start=(i == 0),
                    stop=(i == HT - 1),
                )
            expj = batch.tile([1, 512], BF16, tag="expj", bufs=2)
            nc.scalar.activation(
                expj[:], ps[:], Exp, accum_out=ssum4[0:1, j : j + 1]
            )
            return expj

        def tail_attn(j, expj, eT16, accT):
            # replicate the exp row across partitions with a contraction-1 PE
            # matmul (ones-row weight), then per-partition dot products
            # against enc^T on DVE: accT[p, c, j] = sum_s exp[s]*encT[c,p, s].
            bc = bcp.tile([128, 512], F32, tag="bc")
            nc.tensor.matmul(
                bc[:], ones_bf[0:1, :], expj[0:1, :], start=True, stop=True
            )
            expb = batch.tile([128, 512], BF16, tag="expb", bufs=2)
            nc.vector.tensor_copy(expb[:], bc[:])
            scr = batch.tile([128, HT, 512], BF16, tag="ttr", bufs=2)
            for c in range(HT):
                nc.gpsimd.tensor_tensor(scr[:, c, :], eT16[:, c, :], expb[:], Mult)
            nc.vector.tensor_reduce(accT[:, :, j], scr[:], axis=X, op=Add)

        def finish_batch(b, accT, ssum4):
            ssum = batch.tile([1, 1], F32, tag="ssum")
            nc.vector.reduce_sum(ssum[:], ssum4[:], axis=X)
            inv = batch.tile([1, 1], F32, tag="inv")
            nc.vector.reciprocal(inv[:], ssum[:])
            pb = tp.tile([128, 1], F32, tag="tp", name="pinv")
            nc.tensor.matmul(
                pb[:], ones_f32[0:1, :], inv[0:1, :], start=True, stop=True
            )
            invb = batch.tile([128, 1], F32, tag="invb", bufs=2)
            nc.vector.tensor_copy(invb[:], pb[:])
            oT = batch.tile([128, HT], F32, tag="oT", bufs=2)
            nc.vector.reduce_sum(oT[:], accT[:], axis=X)
            outb = batch.tile([128, HT], F32, tag="outb", bufs=2)
            nc.vector.tensor_scalar_mul(outb[:], oT[:], invb[:])
            nc.gpsimd.dma_start(outT_d[b], outb[:])

        pos = {}
        pending = None
        for b in range(BPC):
            pos[b] = (
                batch.tile([128, HT, SC], F32, tag="accT", name=f"accT_{b}", bufs=2),
                batch.tile([1, SC], F32, tag="ssum4", name=f"ssum4_{b}"),
            )
            for j in range(SC):
                # stage chunk (b+1, j) while computing chunk (b, j)
                if b + 1 < BPC:
                    staged[(b + 1, j)] = load_enc_chunk(b + 1, j)
                last_chunk = b == BPC - 1 and j == SC - 1
                if b == 0 and j == 0:
                    eT8, eT16 = eT8_00, eT16_00
                    pre = pks00
                else:
                    eT8, eT16 = staged.pop((b, j))
                    pre = ()
                p = pending

                def mid(p=p):
                    p["expj"] = tail_scores(p["j"], p["th3"], p["ssum4"])

                eT_j, th3 = kproj_head(
                    b,
                    j,
                    eT8,
                    pks_pre=pre,
                    mid_cb=(mid if p is not None else None),
                    skip_reduce=last_chunk,
                )
                if p is not None:
                    tail_attn(p["j"], p["expj"], p["eT16"], p["accT"])
                    if p["j"] == SC - 1:
                        finish_batch(p["b"], p["accT"], p["ssum4"])
                        pos.pop(p["b"])
                pending = dict(
                    b=b, j=j, th3=th3, eT_j=eT_j, eT16=eT16,
                    accT=pos[b][0], ssum4=pos[b][1],
                )
        # flush the final chunk (direct PE scores path, no DVE dependency)
        p = pending
        expj = tail_scores_direct(p["j"], p["eT_j"], p["ssum4"])
        tail_attn(p["j"], expj, p["eT16"], p["accT"])
        finish_batch(p["b"], p["accT"], p["ssum4"])

    nc.compile()
    return nc


_CACHED_NC = None


def _get_nc():
    global _CACHED_NC
    if _CACHED_NC is None:
        _CACHED_NC = build_program()
    return _CACHED_NC


_F8 = mybir.dt.np(FP8)
_BF = mybir.dt.np(BF16)


def make_in_maps(hidden, encoder_outputs, Wq, bq, Wk, bk, wv):
    """Host-side shard + layout prep (all compute FLOPs stay on device)."""
    hid_last = np.asarray(hidden, np.float32)[-1]  # [32, H]
    enc = np.asarray(encoder_outputs, np.float32)
    Wq = np.asarray(Wq, np.float32)
    Wk = np.asarray(Wk, np.float32)
    bqkT = np.ascontiguousarray(
        (np.asarray(bq, np.float32) + np.asarray(bk, np.float32)).reshape(HT, 128).T
    )
    wvT = np.ascontiguousarray(np.asarray(wv, np.float32).reshape(HT, 128).T)

    # enc^T fp8 tiles: [B, SC, 128(p), HT(c), 512(s)]
    encT = enc.reshape(B, SC, 512, HT, 128).transpose(0, 1, 4, 3, 2)
    encT8 = np.ascontiguousarray(np.clip(encT, -240, 240)).astype(_F8)
    encT16 = np.ascontiguousarray(encT).astype(_BF)

    # Wk^T fp8 with x64 prescale: [128(p), HT(i), HT(c), 128(m)]
    wkT8 = np.ascontiguousarray(
        np.clip(Wk * WK_SCALE, -240, 240)
        .reshape(HT, 128, HT, 128)
        .transpose(3, 0, 2, 1)
    ).astype(_F8)
    # Wq^T bf16: [128(p), HT(t), HT(c), 128(n)]
    wqT16 = np.ascontiguousarray(
        Wq.reshape(HT, 128, HT, 128).transpose(3, 0, 2, 1)
    ).astype(_BF)

    in_maps = []
    for c in range(NCORES):
        sl = slice(c * BPC, (c + 1) * BPC)
        # hid^T: [128(p), HT(c), BPC(b)] = hid[b, 128c+p]
        hidT = np.ascontiguousarray(
            hid_last[sl].reshape(BPC, HT, 128).transpose(2, 1, 0)
        ).astype(_BF)
        in_maps.append(
            {
                "encT8": np.ascontiguousarray(encT8[sl]),
                "encT16": np.ascontiguousarray(encT16[sl]),
                "wkT8": wkT8,
                "wqT16": wqT16,
                "hidT": hidT,
                "bqkT": bqkT,
                "wvT": wvT,
            }
        )
    return in_maps


def run(inputs, trace=False):
    """Run on hardware; returns (output [32,1,1024], BassKernelResults)."""
    nc = _get_nc()
    in_maps = make_in_maps(
        inputs["hidden"],
        inputs["encoder_outputs"],
        inputs["Wq"],
        inputs["bq"],
        inputs["Wk"],
        inputs["bk"],
        inputs["wv"],
    )
    res = run_bass_kernel_spmd(nc, in_maps, list(range(NCORES)), trace=trace)
    outT = np.concatenate([res.results[c]["outT"] for c in range(NCORES)], axis=0)
    # outT[b, p, c] -> out[b, 0, 128c+p]
    out = outT.transpose(0, 2, 1).reshape(B, 1, H)
    return np.ascontiguousarray(out).astype(np.float32), res


def kernel(hidden, encoder_outputs, Wq, bq, Wk, bk, wv, bv):
    out, _ = run(
        {
            "hidden": hidden,
            "encoder_outputs": encoder_outputs,
            "Wq": Wq,
            "bq": bq,
            "Wk": Wk,
            "bk": bk,
            "wv": wv,
        }
    )
    return out


# revision 22
# speedup vs baseline: 1.6272x; 1.6272x over previous
"""Bahdanau attention kernel for 8 Trainium2 NeuronCores.

Problem shapes (hardcoded): hidden [2, 32, 1024], encoder_outputs [32, 2048, 1024],
Wq/Wk [1024, 1024], bq/bk/wv [1024], bv scalar. Output [32, 1, 1024].

Sharding: data-parallel over batch B=32 -> 4 batches per core, weights replicated.
bv is dropped entirely (softmax is invariant to constant shifts).

Key structure (v3):
- The K-projection (enc @ Wk.T, the dominant 137 GFLOP) runs in fp8e4 with
  MatmulPerfMode.DoubleRow (2 fp8 MACs per cell per cycle): 4 accumulating MMs
  of contraction 256 per (o-tile, s-chunk). Wk is pre-scaled by 64 on the host
  so its values sit in fp8's normal range; the inverse scale folds into the
  tanh activation's free scale multiplier.
- All layout/dtype prep happens host-side in make_in_maps (sharding code):
  enc ships twice (pre-transposed fp8 [h, s] tiles for the projection, natural
  bf16 rows for the einsum); Wk^T/Wq^T/hid^T/biases ship pre-transposed. No
  on-device casts or transposes; staging is chunk-granular DMA in deep rings.
- The q+bq+bk bias folds into the tanh as a per-partition bias while the
  activation reads the matmul PSUM directly.
- The wv contraction over h runs on the vector engine (broadcast multiply +
  half-tree sum over the 8 o-tiles), leaving the PE a single ones-weight
  matmul per chunk for the cross-partition sum: the PE's M=1 matmul count
  drops from 256 to 144 per core.
- scores never materialize: exp() is applied per chunk straight from the
  scores PSUM (no max-shift needed: |scores| <= sum|wv| <= 16), and the
  attn @ enc einsum accumulates per chunk with unnormalized weights; only the
  final [1, H] row is scaled by 1/sum.
"""

from contextlib import ExitStack

import numpy as np

import concourse.bacc as bacc
import concourse.bass as bass
import concourse.mybir as mybir
import concourse.tile as tile
from concourse.bass_utils import run_bass_kernel_spmd

B, S, H = 32, 2048, 1024
NCORES = 8
BPC = B // NCORES  # 4 batches per core
F32 = mybir.dt.float32
BF16 = mybir.dt.bfloat16
FP8 = mybir.dt.float8e4
HT = H // 128  # 8 chunks of 128 along h or o
ST = S // 128  # 16 s-tiles of 128
SC = S // 512  # 4 s-chunks of 512
KT = 4  # fp8 DoubleRow: 4 contraction steps of 256
WK_SCALE = 64.0
Tanh = mybir.ActivationFunctionType.Tanh
Exp = mybir.ActivationFunctionType.Exp
X = mybir.AxisListType.X
DR = mybir.MatmulPerfMode.DoubleRow
Mult = mybir.AluOpType.mult

ts = bass.ts


def build_program():
    nc = bacc.Bacc("TRN2", target_bir_lowering=False, debug=False)

    # enc^T fp8 tiles: encT8[b, j, p, c, s] = fp8(enc[b, 512j+s, 128c+p])
    encT8_d = nc.dram_tensor("encT8", [BPC, SC, 128, HT, 512], FP8, kind="ExternalInput")
    # enc natural bf16 rows (einsum operand)
    encN_d = nc.dram_tensor("encN", [BPC, S, H], BF16, kind="ExternalInput")
    # Wk^T fp8 (x64): wkT8[p, i, c, m] = fp8(64 * Wk[128i+m, 128c+p])
    wkT8_d = nc.dram_tensor("wkT8", [128, HT, HT, 128], FP8, kind="ExternalInput")
    # Wq^T bf16: wqT16[p, t, c, n] = bf16(Wq[128t+n, 128c+p])
    wqT16_d = nc.dram_tensor("wqT16", [128, HT, HT, 128], BF16, kind="ExternalInput")
    # hid^T bf16: hidT[p, c, b] = bf16(hidden[-1][4c0+b? no: hid[b, 128c+p]])
    hidT_d = nc.dram_tensor("hidT", [128, HT, BPC], BF16, kind="ExternalInput")
    bqkT_d = nc.dram_tensor("bqkT", [128, HT], F32, kind="ExternalInput")  # (bq+bk)^T
    wvT_d = nc.dram_tensor("wvT", [128, HT], F32, kind="ExternalInput")  # wv^T
    out_d = nc.dram_tensor("out", [BPC, 1, H], F32, kind="ExternalOutput")

    with tile.TileContext(nc) as tc, ExitStack() as ctx:
        consts = ctx.enter_context(tc.tile_pool(name="consts", bufs=1))
        tp = ctx.enter_context(tc.tile_pool(name="tp", bufs=2, space="PSUM"))
        kp = ctx.enter_context(tc.tile_pool(name="kp", bufs=4, space="PSUM"))
        vp = ctx.enter_context(tc.tile_pool(name="vp", bufs=2, space="PSUM"))
        # chunk-granular staging rings
        encT_p = ctx.enter_context(tc.tile_pool(name="encT", bufs=6))  # 512KB/slot
        encbf = ctx.enter_context(tc.tile_pool(name="encbf", bufs=7))  # 1MB/slot
        eT_p = ctx.enter_context(tc.tile_pool(name="eT", bufs=2))
        batch = ctx.enter_context(tc.tile_pool(name="batch", bufs=1))

        # ---- staging helpers (chunk granular, plain HWDGE DMAs) ----
        def load_enc_chunk(b, j):
            eT8 = encT_p.tile([128, HT, 512], FP8, tag="encT8")
            nc.sync.dma_start(eT8[:], encT8_d[b, j])
            eb4 = encbf.tile([128, 4, H], BF16, tag="encbf")
            nc.sync.dma_start(
                eb4[:], encN_d[b, ts(j, 512), :].rearrange("(u p) h -> p u h", p=128)
            )
            return eT8, eb4

        # ---- weights + small consts (pre-transposed on host) ----
        wkT8 = consts.tile([128, HT, HT, 128], FP8, tag="wkT8")
        nc.sync.dma_start(wkT8[:], wkT8_d[:])

        bqkT = consts.tile([128, HT], F32, tag="bqkT")
        nc.scalar.dma_start(bqkT[:], bqkT_d[:])
        wvT = consts.tile([128, HT], F32, tag="wvT")
        nc.scalar.dma_start(wvT[:], wvT_d[:])
        hidT = consts.tile([128, HT, BPC], BF16, tag="hidT")
        nc.scalar.dma_start(hidT[:], hidT_d[:])
        ones_bf = consts.tile([1, 128], BF16, tag="ones")
        nc.vector.memset(ones_bf[:], 1.0)
        ones_col = consts.tile([128, 1], BF16, tag="onescol")
        nc.vector.memset(ones_col[:], 1.0)
        wvT16 = consts.tile([128, HT], BF16, tag="wvT16")
        nc.vector.tensor_copy(wvT16[:], wvT[:])
        wqT16 = consts.tile([128, HT, HT, 128], BF16, tag="wqT16")
        nc.scalar.dma_start(wqT16[:], wqT16_d[:])

        staged = {}
        for j in range(SC):
            staged[(0, j)] = load_enc_chunk(0, j)

        def kproj_mm_chain(i, eT8):
            pk = kp.tile([128, 512], F32, tag="kp", name="pk")
            for kt in range(KT):
                nc.tensor.matmul(
                    pk[:],
                    wkT8[:, i, ts(kt, 2), :],
                    eT8[:, ts(kt, 2), :],
                    start=(kt == 0),
                    stop=(kt == KT - 1),
                    perf_mode=DR,
                )
            return pk

        # ---- chunk (0,0): emit the first 4 kproj MM chains ahead of the
        # q-projection, so the PE starts as soon as wkT8 + the first enc tile
        # land (the q-projection waits on the larger wqT16 transfer).
        eT8_00, eb4_00 = staged.pop((0, 0))
        pks00 = [kproj_mm_chain(i, eT8_00) for i in range(4)]

        # ---- q^T + bq + bk: qkb[o(part), o-chunk t, b] (bf16 MMs, fp32 out;
        # pq lives in the tp pool so the kp ring stays free for pks00) ----
        qkb = consts.tile([128, HT, BPC], F32, tag="qkb")
        for t in range(HT):
            pq = tp.tile([128, BPC], F32, tag="tp", name="pq")
            for c in range(HT):
                nc.tensor.matmul(
                    pq[:],
                    wqT16[:, t, c, :],
                    hidT[:, c, :],
                    start=(c == 0),
                    stop=(c == HT - 1),
                )
            nc.vector.tensor_scalar_add(qkb[:, t, :], pq[:], bqkT[:, t : t + 1])

        # ---- per-batch pieces ----
        # The chunk tail is software-pipelined one chunk behind the
        # K-projection: the scores cross-partition MM + exp for chunk j are
        # emitted mid-way through chunk j+1's kproj (so the exp latency hides
        # under the remaining MM chains), and the attn transposes + einsum
        # follow the full head.
        def kproj_head(b, j, eT8, pks_pre=(), mid_cb=None, skip_reduce=False):
            # K^T tiles (fp8 DoubleRow) + fused bias/tanh (undoes the x64
            # weight prescale) -> eT_j[o(part), o-chunk i, s(512)]
            eT_j = eT_p.tile([128, HT, 512], BF16, tag="eTj")
            for i in range(HT):
                pk = pks_pre[i] if i < len(pks_pre) else kproj_mm_chain(i, eT8)
                nc.scalar.activation(
                    eT_j[:, i, :],
                    pk[:],
                    Tanh,
                    bias=qkb[:, i, b : b + 1],
                    scale=1.0 / WK_SCALE,
                )
                if i == 3 and mid_cb is not None:
                    mid_cb()
            if skip_reduce:
                return eT_j, None

            # scores chunk j = wv . eT_j: per-o-tile multiply by wv^T and
            # half-tree sum over the 8 o-tiles, all on DVE.
            etw = batch.tile([128, HT, 512], BF16, tag="etw", bufs=2)
            for i in range(HT):
                nc.vector.tensor_scalar_mul(
                    etw[:, i, :], eT_j[:, i, :], wvT[:, i : i + 1]
                )
            th1 = batch.tile([128, 4, 512], BF16, tag="th1", bufs=2)
            nc.vector.tensor_add(th1[:], etw[:, 0:4, :], etw[:, 4:8, :])
            th2 = batch.tile([128, 2, 512], BF16, tag="th2", bufs=2)
            nc.vector.tensor_add(th2[:], th1[:, 0:2, :], th1[:, 2:4, :])
            th3 = batch.tile([128, 512], BF16, tag="th3", bufs=2)
            nc.vector.tensor_add(th3[:], th2[:, 0, :], th2[:, 1, :])
            return eT_j, th3

        def tail_scores(j, th3, ssum4):
            # cross-partition sum of the wv-weighted tanh -> scores PSUM row
            ps = kp.tile([1, 512], F32, tag="kp", name="ps")
            nc.tensor.matmul(ps[:], ones_col[:], th3[:], start=True, stop=True)
            expj = batch.tile([1, 512], BF16, tag="expj", bufs=2)
            nc.scalar.activation(
                expj[:], ps[:], Exp, accum_out=ssum4[0:1, j : j + 1]
            )
            return expj

        def tail_scores_direct(j, eT_j, ssum4):
            # final-chunk fast path: contract wv on the PE (no DVE wait)
            ps = kp.tile([1, 512], F32, tag="kp", name="psd")
            for i in range(HT):
                nc.tensor.matmul(
                    ps[:],
                    wvT16[:, i : i + 1],
                    eT_j[:, i, :],
                    start=(i == 0),
                    stop=(i == HT - 1),
                )
            expj = batch.tile([1, 512], BF16, tag="expj", bufs=2)
            nc.scalar.activation(
                expj[:], ps[:], Exp, accum_out=ssum4[0:1, j : j + 1]
            )
            return expj

        def tail_attn(j, expj, eb4, po0, po1):
            # transpose to [s(part), u] columns
            atTj = batch.tile([128, 4], BF16, tag="atTj", bufs=2)
            for u in range(4):
                pa = tp.tile([128, 1], BF16, tag="tp")
                nc.tensor.transpose(pa[:], expj[0:1, ts(u, 128)], ones_bf[0:1, 0:1])
                nc.vector.tensor_copy(atTj[:, u : u + 1], pa[:])

            # partial einsum: accumulate exp-weighted enc rows into po0/po1
            for hc, po in ((0, po0), (1, po1)):
                for u in range(4):
                    nc.tensor.matmul(
                        po[:],
                        atTj[:, u : u + 1],
                        eb4[:, u, ts(hc, 512)],
                        start=(j == 0 and u == 0),
                        stop=(j == SC - 1 and u == 3),
                    )

        def finish_batch(b, po0, po1, ssum4):
            ssum = batch.tile([1, 1], F32, tag="ssum")
            nc.vector.reduce_sum(ssum[:], ssum4[:], axis=X)
            inv = batch.tile([1, 1], F32, tag="inv")
            nc.vector.reciprocal(inv[:], ssum[:])
            outb = batch.tile([1, H], F32, tag="outb", bufs=2)
            nc.vector.tensor_scalar_mul(outb[0:1, ts(0, 512)], po0[:], inv[0:1, 0:1])
            nc.vector.tensor_scalar_mul(outb[0:1, ts(1, 512)], po1[:], inv[0:1, 0:1])
            nc.gpsimd.dma_start(out_d[b], outb[:])

        pos = {}
        pending = None
        for b in range(BPC):
            pos[b] = (
                vp.tile([1, 512], F32, tag="vp", name=f"po0_{b}"),
                vp.tile([1, 512], F32, tag="vp", name=f"po1_{b}"),
                batch.tile([1, SC], F32, tag="ssum4", name=f"ssum4_{b}"),
            )
            for j in range(SC):
                # stage chunk (b+1, j) while computing chunk (b, j)
                if b + 1 < BPC:
                    staged[(b + 1, j)] = load_enc_chunk(b + 1, j)
                last_chunk = b == BPC - 1 and j == SC - 1
                if b == 0 and j == 0:
                    eT8, eb4 = eT8_00, eb4_00
                    pre = pks00
                else:
                    eT8, eb4 = staged.pop((b, j))
                    pre = ()
                p = pending

                def mid(p=p):
                    p["expj"] = tail_scores(p["j"], p["th3"], p["ssum4"])

                eT_j, th3 = kproj_head(
                    b,
                    j,
                    eT8,
                    pks_pre=pre,
                    mid_cb=(mid if p is not None else None),
                    skip_reduce=last_chunk,
                )
                if p is not None:
                    tail_attn(p["j"], p["expj"], p["eb4"], p["po0"], p["po1"])
                    if p["j"] == SC - 1:
                        finish_batch(
                            p["b"], p["po0"], p["po1"], p["ssum4"]
                        )
                        pos.pop(p["b"])
                pending = dict(
                    b=b, j=j, th3=th3, eT_j=eT_j, eb4=eb4,
                    po0=pos[b][0], po1=pos[b][1], ssum4=pos[b][2],
                )
        # flush the final chunk (direct PE scores path, no DVE dependency)
        p = pending
        expj = tail_scores_direct(p["j"], p["eT_j"], p["ssum4"])
        tail_attn(p["j"], expj, p["eb4"], p["po0"], p["po1"])
        finish_batch(p["b"], p["po0"], p["po1"], p["ssum4"])

    nc.compile()
    return nc


_CACHED_NC = None


def _get_nc():
    global _CACHED_NC
    if _CACHED_NC is None:
        _CACHED_NC = build_program()
    return _CACHED_NC


_F8 = mybir.dt.np(FP8)
_BF = mybir.dt.np(BF16)


def make_in_maps(hidden, encoder_outputs, Wq, bq, Wk, bk, wv):
    """Host-side shard + layout prep (all compute FLOPs stay on device)."""
    hid_last = np.asarray(hidden, np.float32)[-1]  # [32, H]
    enc = np.asarray(encoder_outputs, np.float32)
    Wq = np.asarray(Wq, np.float32)
    Wk = np.asarray(Wk, np.float32)
    bqkT = np.ascontiguousarray(
        (np.asarray(bq, np.float32) + np.asarray(bk, np.float32)).reshape(HT, 128).T
    )
    wvT = np.ascontiguousarray(np.asarray(wv, np.float32).reshape(HT, 128).T)

    # enc^T fp8 tiles: [B, SC, 128(p), HT(c), 512(s)]
    encT8 = np.ascontiguousarray(
        np.clip(enc, -240, 240)
        .reshape(B, SC, 512, HT, 128)
        .transpose(0, 1, 4, 3, 2)
    ).astype(_F8)
    encN = enc.astype(_BF)  # natural bf16 rows

    # Wk^T fp8 with x64 prescale: [128(p), HT(i), HT(c), 128(m)]
    wkT8 = np.ascontiguousarray(
        np.clip(Wk * WK_SCALE, -240, 240)
        .reshape(HT, 128, HT, 128)
        .transpose(3, 0, 2, 1)
    ).astype(_F8)
    # Wq^T bf16: [128(p), HT(t), HT(c), 128(n)]
    wqT16 = np.ascontiguousarray(
        Wq.reshape(HT, 128, HT, 128).transpose(3, 0, 2, 1)
    ).astype(_BF)

    in_maps = []
    for c in range(NCORES):
        sl = slice(c * BPC, (c + 1) * BPC)
        # hid^T: [128(p), HT(c), BPC(b)] = hid[b, 128c+p]
        hidT = np.ascontiguousarray(
            hid_last[sl].reshape(BPC, HT, 128).transpose(2, 1, 0)
        ).astype(_BF)
        in_maps.append(
            {
                "encT8": np.ascontiguousarray(encT8[sl]),
                "encN": np.ascontiguousarray(encN[sl]),
                "wkT8": wkT8,
                "wqT16": wqT16,
                "hidT": hidT,
                "bqkT": bqkT,
                "wvT": wvT,
            }
        )
    return in_maps


def run(inputs, trace=False):
    """Run on hardware; returns (output [32,1,1024], BassKernelResults)."""
    nc = _get_nc()
    in_maps = make_in_maps(
        inputs["hidden"],
        inputs["encoder_outputs"],
        inputs["Wq"],
        inputs["bq"],
        inputs["Wk"],
        inputs["bk"],
        inputs["wv"],
    )
    res = run_bass_kernel_spmd(nc, in_maps, list(range(NCORES)), trace=trace)
    out = np.concatenate([res.results[c]["out"] for c in range(NCORES)], axis=0)
    return out.reshape(B, 1, H).astype(np.float32), res


def kernel(hidden, encoder_outputs, Wq, bq, Wk, bk, wv, bv):
    out, _ = run(
        {
            "hidden": hidden,
            "encoder_outputs": encoder_outputs,
            "Wq": Wq,
            "bq": bq,
            "Wk": Wk,
            "bk": bk,
            "wv": wv,
        }
    )
    return out
